# revision 38
# baseline (speedup 1.0000x reference)
"""CPAMDec attention-decoder kernel for 8 Trainium2 NeuronCores.

Reference computation (per batch n of N=8):
    q  = x_n^T @ wq^T + bq          (HW=4096, C4=128)
    k  = y_n @ wk^T + bk            (K=32, C4=128)
    v  = y_n @ wv^T + bv            (K=32, C=512)
    attn = softmax(q @ k^T, axis=-1)        (HW, K)
    out = scale * (v^T @ attn^T) + x_n      (C, HW)

Sharding: pure data parallel - core i computes batch i.

Key optimizations over the fp32 column-streaming baseline:
  1. fp16 wire format. x, out and all weights move over HBM as fp16,
     halving DMA traffic (the serialized-DMA roofline dominates this
     problem: ~26us of transfer at 360 GB/s vs ~52us in fp32). The
     softmax exp/denominator path uses bf16 (energies reach +-12, so
     exp() overflows fp16's 65504 max but fits bf16's range).
  2. q-stage elimination: energy = q.k = (k.wq).x. The tiny kq = k@wq
     matrix ([32,512], built once in the prologue) replaces the
     per-chunk q matmul chain, removing 4 PE matmuls and one
     PSUM->SBUF crossing per chunk. bq folds into a per-key energy
     bias e_b[j] = bq.k[j], applied inside the exp() activation.
  3. No 4x key replication: all matmuls with K=32 contraction read the
     same 32 attn/expt rows (matmul cost depends only on the output
     free size, not the contraction width), so y^T is loaded once and
     the partition-stacked v copies disappear. bv folds into
     v16 = s*(v + bv) via a K=1 ones-row matmul (sum_j attn = 1).
  4. PSUM->SBUF crossings balance across engines: the residual add of
     one output row-tile is a DVE tensor_tensor add straight from
     PSUM; the other three cross via one Scalar activation copy and
     get their residual from an all-fp16 DVE add (2-byte operands hit
     the DVE 2x_1p fast path). The attn normalization multiply lives
     on GpSimd (SBUF-only operands; GPSIMD cannot touch PSUM).
  5. Loads (SP ring) and stores (GpSimd SWDGE ring) stay on separate
     rings so an unready store never head-blocks a load.
  6. The cost model drops the PE clock from 2.4 GHz to 1.2 GHz after
     any idle gap and needs 3us of continuous work to ramp back.
     Dependency-free filler matmuls into a dedicated scratch PSUM
     bank (never read, so never blocked) pad every pipeline step so
     the PE never goes idle and every real matmul runs at full clock.
"""

import sys

sys.path.insert(0, "/opt/trn_rl_repo")

import numpy as np

import concourse.bacc as bacc
import concourse.mybir as mybir
import concourse.tile as tile
from concourse.alu_op_type import AluOpType
from concourse.bass_utils import run_bass_kernel_spmd

F32 = mybir.dt.float32
F16 = mybir.dt.float16
BF16 = mybir.dt.bfloat16
AF = mybir.ActivationFunctionType

N, C, H, W, K = 8, 512, 64, 64, 32
HW = H * W            # 4096
C4 = C // 4           # 128
PC = 512              # free-dim chunk (1 PSUM bank of fp32)
NPC = HW // PC        # 8 chunks
KC = C // 128         # 4 contraction chunks
CT = C // 128         # 4 output row-tiles
NWARM = 8             # PE clock-ramp warmup matmuls
FILL = 0              # per-step PE filler matmuls (PE queue stays backed
#                       up in steady state, so the clock holds by itself)


def _emit(nc, tc):
    sp = nc.sync      # ring for wv0/wk/yt/x loads (SP queue is idle)
    st = nc.sync      # stores ride the same HWDGE ring: every load issue
    #                   precedes the first store emission, so loads are
    #                   never blocked, and HWDGE end-of-kernel drain is
    #                   cheap (SWDGE drain costs ~4us)

    with (
        tc.tile_pool(name="const", bufs=1) as cst,
        tc.tile_pool(name="xbuf", bufs=1) as xp,
        tc.tile_pool(name="work", bufs=3) as wkp,
        tc.tile_pool(name="ps", bufs=2, space="PSUM") as ps,
    ):
        # ------- memsets + act-table preload first: no DMA deps ---------
        # dmw feeds the warmup/filler matmuls (see fill() below); the
        # dummy Exp pays the ~1.3us ACT_TABLE_LOAD before it can hurt.
        dmw = cst.tile([128, PC], F16, name="dmw", tag="dmw")
        nc.vector.memset(dmw[:], 0.0)
        zb = cst.tile([1, 1], F32, name="zb", tag="zb")
        nc.vector.memset(zb[:], 0.0)
        ones32 = cst.tile([K, K], BF16, name="ones32", tag="ones32")
        nc.vector.memset(ones32[:], 1.0)
        ones1 = cst.tile([1, K], F16, name="ones1", tag="ones1")
        nc.vector.memset(ones1[:], 1.0)

        # ---- loads, split across rings so issue time doesn't serialize -
        # SP ring: the prologue-critical weights, then all x chunks.
        xs = [None] * NPC

        def load_chunk(pc):
            t = xp.tile([128, KC, PC], F16, name=f"xs{pc}", tag=f"xs{pc}")
            src = nc.t.x[:, pc * PC:(pc + 1) * PC].rearrange(
                "(k p) f -> p k f", p=128)
            sp.dma_start(t[:], src)
            xs[pc] = t

        wkt, yt = [], []
        for k in range(KC):
            t = cst.tile([128, C4], F16, name=f"wk{k}", tag=f"wk{k}")
            sp.dma_start(t[:], nc.t.wkT[k * 128:(k + 1) * 128, :])
            wkt.append(t)
            t = cst.tile([128, K], F16, name=f"yt{k}", tag=f"yt{k}")
            sp.dma_start(t[:], nc.t.yT[k * 128:(k + 1) * 128, :])
            yt.append(t)
        load_chunk(0)
        load_chunk(1)
        wv = []
        t = cst.tile([128, C], F16, name="wv0", tag="wv0")
        sp.dma_start(t[:], nc.t.wvT[0:128, :])
        wv.append(t)
        for pc in range(2, NPC):
            load_chunk(pc)
        # Scalar ring (queue idle until the first prologue ACT): wq+ident
        wq = cst.tile([C4, C], F16, name="wq", tag="wq")
        nc.scalar.dma_start(wq[:], nc.t.wq[:])
        ident = cst.tile([128, 128], F16, name="ident", tag="ident")
        nc.scalar.dma_start(ident[:], nc.t.ident[:])
        # GpSimd ring (Pool queue idle until the first store): biases+wv
        bk_sb = cst.tile([C4, 1], F32, name="bk_sb", tag="bk_sb")
        nc.gpsimd.dma_start(bk_sb[:], nc.t.bk[:])
        bqb = cst.tile([C4, K], F16, name="bqb", tag="bqb")
        nc.gpsimd.dma_start(bqb[:], nc.t.bqb[:])
        bvr = cst.tile([1, C], F16, name="bvr", tag="bvr")
        nc.gpsimd.dma_start(bvr[:], nc.t.bv[:])
        for k in range(1, KC):
            t = cst.tile([128, C], F16, name=f"wv{k}", tag=f"wv{k}")
            nc.gpsimd.dma_start(t[:], nc.t.wvT[k * 128:(k + 1) * 128, :])
            wv.append(t)
        s_bc = cst.tile([K, 1], F32, name="s_bc", tag="s_bc")
        nc.gpsimd.dma_start(
            s_bc[:], nc.t.s[:].partition_broadcast(K).squeeze(-1))

        # preload the activation table off the critical path (the first
        # ACT instruction pays a ~1.3us ACT_TABLE_LOAD)
        scr = cst.tile([1, 1], F32, name="scr", tag="scr")
        nc.scalar.activation(out=scr[:], in_=zb[:], func=AF.Exp,
                             bias=zb[:], scale=1.0)

        # ------- PE warmup + fillers: dep-free matmuls on zeros ---------
        # dmy_ps is never read, so these matmuls dispatch the moment the
        # PE is free - they start the clock ramp immediately and later
        # plug pipeline bubbles so the PE never drops out of 2.4 GHz.
        dmy_ps = ps.tile([128, PC], F32, name="dmy_ps", tag="dmy", bufs=1)

        def fill(n):
            for _ in range(n):
                nc.tensor.matmul(dmy_ps[:], dmw[:, 0:128], dmw[:],
                                 start=True, stop=True)

        fill(NWARM)

        # ---------------- prologue: ktb, kq^T, e_b, v16 -----------------
        pro = {}

        def prologue_kq():
            # ktb[o, j] = k[j, o] = sum_c wk[o,c] y[j,c] + bk[o]
            kt_ps = ps.tile([C4, K], F32, name="kt_ps", tag="es", bufs=3)
            for k in range(KC):
                nc.tensor.matmul(kt_ps[:], wkt[k][:], yt[k][:],
                                 start=(k == 0), stop=(k == KC - 1))
            ktb = cst.tile([C4, K], F16, name="ktb", tag="ktb")
            nc.scalar.activation(out=ktb[:], in_=kt_ps[:], func=AF.Identity,
                                 bias=bk_sb[:], scale=1.0)
            fill(3)   # cover the PE's wait for the ktb crossing
            # kqT[c', kc, j] = kq[j, 128*kc+c'] = sum_o wq[o,c] k[j,o]
            kq_ps = ps.tile([128, KC, K], F32, name="kq_ps", tag="es",
                            bufs=3)
            for k in range(KC):
                nc.tensor.matmul(kq_ps[:, k, :],
                                 wq[:, k * 128:(k + 1) * 128], ktb[:],
                                 start=True, stop=True)
            kqT = cst.tile([128, KC, K], F16, name="kqT", tag="kqT")
            nc.scalar.activation(out=kqT[:], in_=kq_ps[:], func=AF.Copy,
                                 scale=1.0)
            # per-key energy bias e_b[j] = sum_o bq[o] k[j,o]
            eb_ps = ps.tile([K, K], F32, name="eb_ps", tag="es", bufs=3)
            nc.tensor.matmul(eb_ps[:], ktb[:], bqb[:], start=True, stop=True)
            e_b = cst.tile([K, 1], F32, name="e_b", tag="e_b")
            nc.scalar.activation(out=e_b[:], in_=eb_ps[:, 0:1], func=AF.Copy,
                                 scale=1.0)
            pro.update(ktb=ktb, kqT=kqT, e_b=e_b)

        def prologue_v():
            # v16[j, c] = s * (sum_cc y[j,cc] wv[c,cc] + bv[c])
            v_ps = ps.tile([K, C], F32, name="v_ps", tag="es", bufs=3)
            for k in range(KC):
                nc.tensor.matmul(v_ps[:], yt[k][:], wv[k][:],
                                 start=(k == 0), stop=False)
            nc.tensor.matmul(v_ps[:], ones1[:], bvr[:], start=False,
                             stop=True)
            v16 = cst.tile([K, C], F16, name="v16", tag="v16")
            nc.scalar.activation(out=v16[:], in_=v_ps[:], func=AF.Copy,
                                 bias=0.0, scale=s_bc[:])
            pro.update(v16=v16)

        # ------------- software-pipelined main loop over column chunks --
        expts = [None] * NPC
        attns = [None] * NPC

        def stage_e(pc):
            e_ps = ps.tile([K, PC], F32, name=f"e_ps{pc}", tag="es", bufs=3)
            for k in range(KC):
                nc.tensor.matmul(e_ps[:], pro['kqT'][:, k, :],
                                 xs[pc][:, k, :],
                                 start=(k == 0), stop=(k == KC - 1))
            ex = wkp.tile([K, PC], BF16, name="ex", tag="ex", bufs=4)
            nc.scalar.activation(out=ex[:], in_=e_ps[:], func=AF.Exp,
                                 bias=pro['e_b'][:], scale=1.0)
            expts[pc] = ex

        def stage_sm(pc):
            s_ps = ps.tile([K, PC], F32, name=f"s_ps{pc}", tag="es", bufs=3)
            nc.tensor.matmul(s_ps[:], ones32[:], expts[pc][:],
                             start=True, stop=True)
            rec = wkp.tile([K, PC], F32, name="rec", tag="rec", bufs=2)
            nc.vector.reciprocal_approx_fast(out=rec[:], in_=s_ps[:])
            at = wkp.tile([K, PC], F16, name="at", tag="at", bufs=3)
            # alternate the normalization multiply between DVE and Pool
            # to balance the two queues (Pool also issues the stores)
            eng = nc.vector if pc % 2 == 0 else nc.gpsimd
            eng.tensor_mul(at[:], expts[pc][:], rec[:])
            attns[pc] = at

        def stage_out(pc):
            # two 2-bank PSUM pair-tiles per chunk sharing a
            # double-buffered tag, so the next chunk's matmuls never wait
            # on this chunk's crossings.
            osb = wkp.tile([128, CT, PC], F16, name="osb", tag="osb", bufs=3)
            # pair A (row-tiles 0,1): the residual x is accumulated on
            # the PE by an identity matmul, so the crossing is a pure
            # Scalar cast
            oa = ps.tile([128, 2, PC], F32, name=f"oa{pc}", tag="o", bufs=2)
            for i in range(2):
                nc.tensor.matmul(oa[:, i, :],
                                 pro['v16'][:, i * 128:(i + 1) * 128],
                                 attns[pc][:], start=True, stop=False)
                nc.tensor.matmul(oa[:, i, :], ident[:], xs[pc][:, i, :],
                                 start=False, stop=True)
            nc.scalar.activation(out=osb[:, 0:2, :], in_=oa[:],
                                 func=AF.Copy, scale=1.0)
            # pair B (row-tiles 2,3): plain matmuls, residual added by
            # the DVE straight from PSUM
            obp = ps.tile([128, 2, PC], F32, name=f"ob{pc}", tag="o",
                          bufs=2)
            for i in range(2):
                ct = 2 + i
                nc.tensor.matmul(obp[:, i, :],
                                 pro['v16'][:, ct * 128:(ct + 1) * 128],
                                 attns[pc][:], start=True, stop=True)
            nc.vector.tensor_add(osb[:, 2:, :], obp[:], xs[pc][:, 2:, :])
            dst = nc.t.out[:, pc * PC:(pc + 1) * PC].rearrange(
                "(k p) f -> p k f", p=128)
            st.dma_start(dst, osb[:])

        prologue_kq()
        prologue_v()
        stage_e(0)
        # skew: exp rides with e; softmax 2 steps behind; out 4 behind.
        # Every cross-engine dependency gets >=1 full step of slack, so
        # the in-order queues never head-block each other.
        for step in range(1, NPC + 4):
            fill(FILL if step < NPC else 2)
            if 0 <= step - 2 < NPC:
                stage_sm(step - 2)
            if 0 <= step - 3 < NPC:
                stage_out(step - 3)
            if step < NPC:
                stage_e(step)


class _T:
    """Attribute access to declared dram params."""
    def __init__(self):
        self.__dict__ = {}


_NC_CACHE = []


def _build():
    if _NC_CACHE:
        return _NC_CACHE[0]
    nc = bacc.Bacc(target_bir_lowering=False)
    nc.t = _T()
    t = nc.t
    t.x = nc.declare_dram_parameter("x", [C, HW], F16, isOutput=False)
    t.yT = nc.declare_dram_parameter("yT", [C, K], F16, isOutput=False)
    t.wq = nc.declare_dram_parameter("wq", [C4, C], F16, isOutput=False)
    t.wkT = nc.declare_dram_parameter("wkT", [C, C4], F16, isOutput=False)
    t.wvT = nc.declare_dram_parameter("wvT", [C, C], F16, isOutput=False)
    t.bqb = nc.declare_dram_parameter("bqb", [C4, K], F16, isOutput=False)
    t.bk = nc.declare_dram_parameter("bk", [C4, 1], F32, isOutput=False)
    t.bv = nc.declare_dram_parameter("bv", [1, C], F16, isOutput=False)
    t.s = nc.declare_dram_parameter("s", [1, 1], F32, isOutput=False)
    t.ident = nc.declare_dram_parameter("ident", [128, 128], F16,
                                        isOutput=False)
    t.out = nc.declare_dram_parameter("out", [C, HW], F16, isOutput=True)
    with tile.TileContext(nc) as tc:
        _emit(nc, tc)
    nc.finalize()
    _NC_CACHE.append(nc)
    return nc


def _in_maps(x, y, wq, bq, wk, bk, wv, bv, scale):
    x16 = np.ascontiguousarray(x, dtype=np.float16).reshape(N, C, HW)
    yT = np.ascontiguousarray(
        np.transpose(y, (0, 2, 1)), dtype=np.float16)
    wq16 = np.ascontiguousarray(wq, dtype=np.float16)
    wkT = np.ascontiguousarray(np.float16(wk).T)
    wvT = np.ascontiguousarray(np.float16(wv).T)
    bqb = np.ascontiguousarray(
        np.broadcast_to(np.float16(bq).reshape(C4, 1), (C4, K)))
    bk32 = np.ascontiguousarray(bk, dtype=np.float32).reshape(C4, 1)
    bv16 = np.ascontiguousarray(bv, dtype=np.float16).reshape(1, C)
    s = np.ascontiguousarray(scale, dtype=np.float32).reshape(1, 1)
    ident = np.eye(128, dtype=np.float16)
    return [
        {
            "x": x16[i], "yT": yT[i], "wq": wq16, "wkT": wkT, "wvT": wvT,
            "bqb": bqb, "bk": bk32, "bv": bv16, "s": s, "ident": ident,
        }
        for i in range(N)
    ]


def _run(inputs, **kwargs):
    nc = _build()
    return run_bass_kernel_spmd(nc, _in_maps(**inputs),
                                core_ids=list(range(N)), **kwargs)


def kernel(**inputs) -> np.ndarray:
    res = _run(inputs)
    out = np.stack([res.results[i]["out"] for i in range(N)])
    return out.reshape(N, C, H, W).astype(np.float32)


# revision 39
# speedup vs baseline: 1.0342x; 1.0342x over previous
"""CPAMDec attention-decoder kernel for 8 Trainium2 NeuronCores.

Reference computation (per batch n of N=8):
    q  = x_n^T @ wq^T + bq          (HW=4096, C4=128)
    k  = y_n @ wk^T + bk            (K=32, C4=128)
    v  = y_n @ wv^T + bv            (K=32, C=512)
    attn = softmax(q @ k^T, axis=-1)        (HW, K)
    out = scale * (v^T @ attn^T) + x_n      (C, HW)

Sharding: pure data parallel - core i computes batch i.

Key optimizations over the fp32 column-streaming baseline:
  1. fp16 wire format. x, out and all weights move over HBM as fp16,
     halving DMA traffic (the serialized-DMA roofline dominates this
     problem: ~26us of transfer at 360 GB/s vs ~52us in fp32). The
     softmax exp/denominator path uses bf16 (energies reach +-12, so
     exp() overflows fp16's 65504 max but fits bf16's range).
  2. q-stage elimination: energy = q.k = (k.wq).x. The tiny kq = k@wq
     matrix ([32,512], built once in the prologue) replaces the
     per-chunk q matmul chain, removing 4 PE matmuls and one
     PSUM->SBUF crossing per chunk. bq folds into a per-key energy
     bias e_b[j] = bq.k[j], applied inside the exp() activation.
  3. No 4x key replication: all matmuls with K=32 contraction read the
     same 32 attn/expt rows (matmul cost depends only on the output
     free size, not the contraction width), so y^T is loaded once and
     the partition-stacked v copies disappear. bv folds into
     v16 = s*(v + bv) via a K=1 ones-row matmul (sum_j attn = 1).
  4. PSUM->SBUF crossings balance across engines: the residual add of
     one output row-tile is a DVE tensor_tensor add straight from
     PSUM; the other three cross via one Scalar activation copy and
     get their residual from an all-fp16 DVE add (2-byte operands hit
     the DVE 2x_1p fast path). The attn normalization multiply lives
     on GpSimd (SBUF-only operands; GPSIMD cannot touch PSUM).
  5. Loads (SP ring) and stores (GpSimd SWDGE ring) stay on separate
     rings so an unready store never head-blocks a load.
  6. The cost model drops the PE clock from 2.4 GHz to 1.2 GHz after
     any idle gap and needs 3us of continuous work to ramp back.
     Dependency-free filler matmuls into a dedicated scratch PSUM
     bank (never read, so never blocked) pad every pipeline step so
     the PE never goes idle and every real matmul runs at full clock.
"""

import sys

sys.path.insert(0, "/opt/trn_rl_repo")

import numpy as np

import concourse.bacc as bacc
import concourse.mybir as mybir
import concourse.tile as tile
from concourse.alu_op_type import AluOpType
from concourse.bass_utils import run_bass_kernel_spmd

F32 = mybir.dt.float32
F16 = mybir.dt.float16
BF16 = mybir.dt.bfloat16
AF = mybir.ActivationFunctionType

N, C, H, W, K = 8, 512, 64, 64, 32
HW = H * W            # 4096
C4 = C // 4           # 128
PC = 512              # free-dim chunk (1 PSUM bank of fp32)
NPC = HW // PC        # 8 chunks
KC = C // 128         # 4 contraction chunks
CT = C // 128         # 4 output row-tiles
NWARM = 14            # PE clock-ramp warmup matmuls
FILL = 1              # per-step PE filler matmuls (keep the clock hot)


def _emit(nc, tc):
    sp = nc.sync      # ring for wv0/wk/yt/x loads (SP queue is idle)
    st = nc.sync      # stores ride the same HWDGE ring: every load issue
    #                   precedes the first store emission, so loads are
    #                   never blocked, and HWDGE end-of-kernel drain is
    #                   cheap (SWDGE drain costs ~4us)

    with (
        tc.tile_pool(name="const", bufs=1) as cst,
        tc.tile_pool(name="xbuf", bufs=1) as xp,
        tc.tile_pool(name="work", bufs=3) as wkp,
        tc.tile_pool(name="ps", bufs=2, space="PSUM") as ps,
    ):
        # ------- memsets + act-table preload first: no DMA deps ---------
        # dmw feeds the warmup/filler matmuls (see fill() below); the
        # dummy Exp pays the ~1.3us ACT_TABLE_LOAD before it can hurt.
        dmw = cst.tile([128, PC], F16, name="dmw", tag="dmw")
        nc.vector.memset(dmw[:], 0.0)
        zb = cst.tile([1, 1], F32, name="zb", tag="zb")
        nc.vector.memset(zb[:], 0.0)
        ones32 = cst.tile([K, K], BF16, name="ones32", tag="ones32")
        nc.vector.memset(ones32[:], 1.0)
        ones1 = cst.tile([1, K], F16, name="ones1", tag="ones1")
        nc.vector.memset(ones1[:], 1.0)

        # ---- loads, split across rings so issue time doesn't serialize -
        # SP ring: the prologue-critical weights, then all x chunks.
        xs = [None] * NPC

        def load_chunk(pc):
            t = xp.tile([128, KC, PC], F16, name=f"xs{pc}", tag=f"xs{pc}")
            src = nc.t.x[:, pc * PC:(pc + 1) * PC].rearrange(
                "(k p) f -> p k f", p=128)
            sp.dma_start(t[:], src)
            xs[pc] = t

        wkt, yt = [], []
        for k in range(KC):
            t = cst.tile([128, C4], F16, name=f"wk{k}", tag=f"wk{k}")
            sp.dma_start(t[:], nc.t.wkT[k * 128:(k + 1) * 128, :])
            wkt.append(t)
            t = cst.tile([128, K], F16, name=f"yt{k}", tag=f"yt{k}")
            sp.dma_start(t[:], nc.t.yT[k * 128:(k + 1) * 128, :])
            yt.append(t)
        load_chunk(0)
        load_chunk(1)
        wv = []
        t = cst.tile([128, C], F16, name="wv0", tag="wv0")
        sp.dma_start(t[:], nc.t.wvT[0:128, :])
        wv.append(t)
        for pc in range(2, NPC):
            load_chunk(pc)
        # Scalar ring (queue idle until the first prologue ACT): wq+ident
        wq = cst.tile([C4, C], F16, name="wq", tag="wq")
        nc.scalar.dma_start(wq[:], nc.t.wq[:])
        ident = cst.tile([128, 128], F16, name="ident", tag="ident")
        nc.scalar.dma_start(ident[:], nc.t.ident[:])
        # GpSimd ring (Pool queue idle until the first store): biases+wv
        bk_sb = cst.tile([C4, 1], F32, name="bk_sb", tag="bk_sb")
        nc.gpsimd.dma_start(bk_sb[:], nc.t.bk[:])
        bqb = cst.tile([C4, K], F16, name="bqb", tag="bqb")
        nc.gpsimd.dma_start(bqb[:], nc.t.bqb[:])
        bvr = cst.tile([1, C], F16, name="bvr", tag="bvr")
        nc.gpsimd.dma_start(bvr[:], nc.t.bv[:])
        for k in range(1, KC):
            t = cst.tile([128, C], F16, name=f"wv{k}", tag=f"wv{k}")
            nc.gpsimd.dma_start(t[:], nc.t.wvT[k * 128:(k + 1) * 128, :])
            wv.append(t)
        s_bc = cst.tile([K, 1], F32, name="s_bc", tag="s_bc")
        nc.gpsimd.dma_start(
            s_bc[:], nc.t.s[:].partition_broadcast(K).squeeze(-1))

        # preload the activation table off the critical path (the first
        # ACT instruction pays a ~1.3us ACT_TABLE_LOAD)
        scr = cst.tile([1, 1], F32, name="scr", tag="scr")
        nc.scalar.activation(out=scr[:], in_=zb[:], func=AF.Exp,
                             bias=zb[:], scale=1.0)

        # ------- PE warmup + fillers: dep-free matmuls on zeros ---------
        # dmy_ps is never read, so these matmuls dispatch the moment the
        # PE is free - they start the clock ramp immediately and later
        # plug pipeline bubbles so the PE never drops out of 2.4 GHz.
        dmy_ps = ps.tile([128, PC], F32, name="dmy_ps", tag="dmy", bufs=1)

        def fill(n):
            for _ in range(n):
                nc.tensor.matmul(dmy_ps[:], dmw[:, 0:128], dmw[:],
                                 start=True, stop=True)

        fill(NWARM)

        # ---------------- prologue: ktb, kq^T, e_b, v16 -----------------
        pro = {}

        def prologue_kq():
            # ktb[o, j] = k[j, o] = sum_c wk[o,c] y[j,c] + bk[o]
            kt_ps = ps.tile([C4, K], F32, name="kt_ps", tag="es", bufs=3)
            for k in range(KC):
                nc.tensor.matmul(kt_ps[:], wkt[k][:], yt[k][:],
                                 start=(k == 0), stop=(k == KC - 1))
            ktb = cst.tile([C4, K], F16, name="ktb", tag="ktb")
            nc.scalar.activation(out=ktb[:], in_=kt_ps[:], func=AF.Identity,
                                 bias=bk_sb[:], scale=1.0)
            fill(3)   # cover the PE's wait for the ktb crossing
            # kqT[c', kc, j] = kq[j, 128*kc+c'] = sum_o wq[o,c] k[j,o]
            kq_ps = ps.tile([128, KC, K], F32, name="kq_ps", tag="es",
                            bufs=3)
            for k in range(KC):
                nc.tensor.matmul(kq_ps[:, k, :],
                                 wq[:, k * 128:(k + 1) * 128], ktb[:],
                                 start=True, stop=True)
            kqT = cst.tile([128, KC, K], F16, name="kqT", tag="kqT")
            nc.scalar.activation(out=kqT[:], in_=kq_ps[:], func=AF.Copy,
                                 scale=1.0)
            # per-key energy bias e_b[j] = sum_o bq[o] k[j,o]
            eb_ps = ps.tile([K, K], F32, name="eb_ps", tag="es", bufs=3)
            nc.tensor.matmul(eb_ps[:], ktb[:], bqb[:], start=True, stop=True)
            e_b = cst.tile([K, 1], F32, name="e_b", tag="e_b")
            nc.scalar.activation(out=e_b[:], in_=eb_ps[:, 0:1], func=AF.Copy,
                                 scale=1.0)
            pro.update(ktb=ktb, kqT=kqT, e_b=e_b)

        def prologue_v():
            # v16[j, c] = s * (sum_cc y[j,cc] wv[c,cc] + bv[c])
            v_ps = ps.tile([K, C], F32, name="v_ps", tag="es", bufs=3)
            for k in range(KC):
                nc.tensor.matmul(v_ps[:], yt[k][:], wv[k][:],
                                 start=(k == 0), stop=False)
            nc.tensor.matmul(v_ps[:], ones1[:], bvr[:], start=False,
                             stop=True)
            v16 = cst.tile([K, C], F16, name="v16", tag="v16")
            nc.scalar.activation(out=v16[:], in_=v_ps[:], func=AF.Copy,
                                 bias=0.0, scale=s_bc[:])
            pro.update(v16=v16)

        # ------------- software-pipelined main loop over column chunks --
        expts = [None] * NPC
        attns = [None] * NPC

        def stage_e(pc):
            e_ps = ps.tile([K, PC], F32, name=f"e_ps{pc}", tag="es", bufs=3)
            for k in range(KC):
                nc.tensor.matmul(e_ps[:], pro['kqT'][:, k, :],
                                 xs[pc][:, k, :],
                                 start=(k == 0), stop=(k == KC - 1))
            ex = wkp.tile([K, PC], BF16, name="ex", tag="ex", bufs=4)
            nc.scalar.activation(out=ex[:], in_=e_ps[:], func=AF.Exp,
                                 bias=pro['e_b'][:], scale=1.0)
            expts[pc] = ex

        def stage_sm(pc):
            s_ps = ps.tile([K, PC], F32, name=f"s_ps{pc}", tag="es", bufs=3)
            nc.tensor.matmul(s_ps[:], ones32[:], expts[pc][:],
                             start=True, stop=True)
            rec = wkp.tile([K, PC], F32, name="rec", tag="rec", bufs=2)
            nc.vector.reciprocal_approx_fast(out=rec[:], in_=s_ps[:])
            at = wkp.tile([K, PC], F16, name="at", tag="at", bufs=3)
            # alternate the normalization multiply between DVE and Pool
            # to balance the two queues (Pool also issues the stores)
            eng = nc.vector if pc % 2 == 0 else nc.gpsimd
            eng.tensor_mul(at[:], expts[pc][:], rec[:])
            attns[pc] = at

        def stage_out(pc):
            # two 2-bank PSUM pair-tiles per chunk sharing a
            # double-buffered tag, so the next chunk's matmuls never wait
            # on this chunk's crossings.
            osb = wkp.tile([128, CT, PC], F16, name="osb", tag="osb", bufs=3)
            # pair A (row-tiles 0,1): the residual x is accumulated on
            # the PE by an identity matmul, so the crossing is a pure
            # Scalar cast
            oa = ps.tile([128, 2, PC], F32, name=f"oa{pc}", tag="o", bufs=2)
            for i in range(2):
                nc.tensor.matmul(oa[:, i, :],
                                 pro['v16'][:, i * 128:(i + 1) * 128],
                                 attns[pc][:], start=True, stop=False)
                nc.tensor.matmul(oa[:, i, :], ident[:], xs[pc][:, i, :],
                                 start=False, stop=True)
            nc.scalar.activation(out=osb[:, 0:2, :], in_=oa[:],
                                 func=AF.Copy, scale=1.0)
            # pair B (row-tiles 2,3): plain matmuls, residual added by
            # the DVE straight from PSUM
            obp = ps.tile([128, 2, PC], F32, name=f"ob{pc}", tag="o",
                          bufs=2)
            for i in range(2):
                ct = 2 + i
                nc.tensor.matmul(obp[:, i, :],
                                 pro['v16'][:, ct * 128:(ct + 1) * 128],
                                 attns[pc][:], start=True, stop=True)
            nc.vector.tensor_add(osb[:, 2:, :], obp[:], xs[pc][:, 2:, :])
            dst = nc.t.out[:, pc * PC:(pc + 1) * PC].rearrange(
                "(k p) f -> p k f", p=128)
            st.dma_start(dst, osb[:])

        prologue_kq()
        prologue_v()
        stage_e(0)
        # skew: exp rides with e; softmax 2 steps behind; out 4 behind.
        # Every cross-engine dependency gets >=1 full step of slack, so
        # the in-order queues never head-block each other.
        for step in range(1, NPC + 4):
            fill(FILL if step < NPC else 2)
            if 0 <= step - 2 < NPC:
                stage_sm(step - 2)
            if 0 <= step - 3 < NPC:
                stage_out(step - 3)
            if step < NPC:
                stage_e(step)


class _T:
    """Attribute access to declared dram params."""
    def __init__(self):
        self.__dict__ = {}


_NC_CACHE = []


def _build():
    if _NC_CACHE:
        return _NC_CACHE[0]
    nc = bacc.Bacc(target_bir_lowering=False)
    nc.t = _T()
    t = nc.t
    t.x = nc.declare_dram_parameter("x", [C, HW], F16, isOutput=False)
    t.yT = nc.declare_dram_parameter("yT", [C, K], F16, isOutput=False)
    t.wq = nc.declare_dram_parameter("wq", [C4, C], F16, isOutput=False)
    t.wkT = nc.declare_dram_parameter("wkT", [C, C4], F16, isOutput=False)
    t.wvT = nc.declare_dram_parameter("wvT", [C, C], F16, isOutput=False)
    t.bqb = nc.declare_dram_parameter("bqb", [C4, K], F16, isOutput=False)
    t.bk = nc.declare_dram_parameter("bk", [C4, 1], F32, isOutput=False)
    t.bv = nc.declare_dram_parameter("bv", [1, C], F16, isOutput=False)
    t.s = nc.declare_dram_parameter("s", [1, 1], F32, isOutput=False)
    t.ident = nc.declare_dram_parameter("ident", [128, 128], F16,
                                        isOutput=False)
    t.out = nc.declare_dram_parameter("out", [C, HW], F16, isOutput=True)
    with tile.TileContext(nc) as tc:
        _emit(nc, tc)
    nc.finalize()
    _NC_CACHE.append(nc)
    return nc


def _in_maps(x, y, wq, bq, wk, bk, wv, bv, scale):
    x16 = np.ascontiguousarray(x, dtype=np.float16).reshape(N, C, HW)
    yT = np.ascontiguousarray(
        np.transpose(y, (0, 2, 1)), dtype=np.float16)
    wq16 = np.ascontiguousarray(wq, dtype=np.float16)
    wkT = np.ascontiguousarray(np.float16(wk).T)
    wvT = np.ascontiguousarray(np.float16(wv).T)
    bqb = np.ascontiguousarray(
        np.broadcast_to(np.float16(bq).reshape(C4, 1), (C4, K)))
    bk32 = np.ascontiguousarray(bk, dtype=np.float32).reshape(C4, 1)
    bv16 = np.ascontiguousarray(bv, dtype=np.float16).reshape(1, C)
    s = np.ascontiguousarray(scale, dtype=np.float32).reshape(1, 1)
    ident = np.eye(128, dtype=np.float16)
    return [
        {
            "x": x16[i], "yT": yT[i], "wq": wq16, "wkT": wkT, "wvT": wvT,
            "bqb": bqb, "bk": bk32, "bv": bv16, "s": s, "ident": ident,
        }
        for i in range(N)
    ]


def _run(inputs, **kwargs):
    nc = _build()
    return run_bass_kernel_spmd(nc, _in_maps(**inputs),
                                core_ids=list(range(N)), **kwargs)


def kernel(**inputs) -> np.ndarray:
    res = _run(inputs)
    out = np.stack([res.results[i]["out"] for i in range(N)])
    return out.reshape(N, C, H, W).astype(np.float32)


# revision 40
# speedup vs baseline: 1.1789x; 1.1399x over previous
"""CPAMDec attention-decoder kernel for 8 Trainium2 NeuronCores.

Reference computation (per batch n of N=8):
    q  = x_n^T @ wq^T + bq          (HW=4096, C4=128)
    k  = y_n @ wk^T + bk            (K=32, C4=128)
    v  = y_n @ wv^T + bv            (K=32, C=512)
    attn = softmax(q @ k^T, axis=-1)        (HW, K)
    out = scale * (v^T @ attn^T) + x_n      (C, HW)

Sharding: pure data parallel - core i computes batch i.

Key optimizations over the fp32 column-streaming baseline:
  1. fp16 wire format. x, out and all weights move over HBM as fp16,
     halving DMA traffic (the serialized-DMA roofline dominates this
     problem: ~26us of transfer at 360 GB/s vs ~52us in fp32). The
     softmax exp/denominator path uses bf16 (energies reach +-12, so
     exp() overflows fp16's 65504 max but fits bf16's range).
  2. q-stage elimination: energy = q.k = (k.wq).x. The tiny kq = k@wq
     matrix ([32,512], built once in the prologue) replaces the
     per-chunk q matmul chain, removing 4 PE matmuls and one
     PSUM->SBUF crossing per chunk. bq folds into a per-key energy
     bias e_b[j] = bq.k[j], applied inside the exp() activation.
  3. No 4x key replication: all matmuls with K=32 contraction read the
     same 32 attn/expt rows (matmul cost depends only on the output
     free size, not the contraction width), so y^T is loaded once and
     the partition-stacked v copies disappear. bv folds into
     v16 = s*(v + bv) via a K=1 ones-row matmul (sum_j attn = 1).
  4. PSUM->SBUF crossings balance across engines: the residual add of
     one output row-tile is a DVE tensor_tensor add straight from
     PSUM; the other three cross via one Scalar activation copy and
     get their residual from an all-fp16 DVE add (2-byte operands hit
     the DVE 2x_1p fast path). The attn normalization multiply lives
     on GpSimd (SBUF-only operands; GPSIMD cannot touch PSUM).
  5. Loads (SP ring) and stores (GpSimd SWDGE ring) stay on separate
     rings so an unready store never head-blocks a load.
  6. The cost model drops the PE clock from 2.4 GHz to 1.2 GHz after
     any idle gap and needs 3us of continuous work to ramp back.
     Dependency-free filler matmuls into a dedicated scratch PSUM
     bank (never read, so never blocked) pad every pipeline step so
     the PE never goes idle and every real matmul runs at full clock.
"""

import sys

sys.path.insert(0, "/opt/trn_rl_repo")

import numpy as np

import concourse.bacc as bacc
import concourse.mybir as mybir
import concourse.tile as tile
from concourse.alu_op_type import AluOpType
from concourse.bass_utils import run_bass_kernel_spmd

F32 = mybir.dt.float32
F16 = mybir.dt.float16
BF16 = mybir.dt.bfloat16
AF = mybir.ActivationFunctionType

N, C, H, W, K = 8, 512, 64, 64, 32
HW = H * W            # 4096
C4 = C // 4           # 128
PC = 512              # free-dim chunk (1 PSUM bank of fp32)
NPC = HW // PC        # 8 chunks
KC = C // 128         # 4 contraction chunks
CT = C // 128         # 4 output row-tiles
NWARM = 14            # PE clock-ramp warmup matmuls
FILL = 1              # per-step PE filler matmuls (keep the clock hot)


def _emit(nc, tc):
    sp = nc.sync      # ring for wv0/wk/yt/x loads (SP queue is idle)
    st = nc.sync      # stores ride the same HWDGE ring: every load issue
    #                   precedes the first store emission, so loads are
    #                   never blocked, and HWDGE end-of-kernel drain is
    #                   cheap (SWDGE drain costs ~4us)

    with (
        tc.tile_pool(name="const", bufs=1) as cst,
        tc.tile_pool(name="xbuf", bufs=1) as xp,
        tc.tile_pool(name="work", bufs=3) as wkp,
        tc.tile_pool(name="ps", bufs=2, space="PSUM") as ps,
    ):
        # ------- memsets + act-table preload first: no DMA deps ---------
        # dmw feeds the warmup/filler matmuls (see fill() below); the
        # dummy Exp pays the ~1.3us ACT_TABLE_LOAD before it can hurt.
        dmw = cst.tile([128, PC], F16, name="dmw", tag="dmw")
        nc.vector.memset(dmw[:], 0.0)
        zb = cst.tile([1, 1], F32, name="zb", tag="zb")
        nc.vector.memset(zb[:], 0.0)
        ones32 = cst.tile([K, K], BF16, name="ones32", tag="ones32")
        nc.vector.memset(ones32[:], 1.0)
        ones1 = cst.tile([1, K], F16, name="ones1", tag="ones1")
        nc.vector.memset(ones1[:], 1.0)

        # ---- loads, split across rings so issue time doesn't serialize -
        # SP ring: the prologue-critical weights, then all x chunks.
        xs = [None] * NPC

        def load_chunk(pc):
            t = xp.tile([128, KC, PC], F16, name=f"xs{pc}", tag=f"xs{pc}")
            src = nc.t.x[:, pc * PC:(pc + 1) * PC].rearrange(
                "(k p) f -> p k f", p=128)
            sp.dma_start(t[:], src)
            xs[pc] = t

        wkt, yt = [], []
        for k in range(KC):
            t = cst.tile([128, C4], F16, name=f"wk{k}", tag=f"wk{k}")
            sp.dma_start(t[:], nc.t.wkT[k * 128:(k + 1) * 128, :])
            wkt.append(t)
            t = cst.tile([128, K], F16, name=f"yt{k}", tag=f"yt{k}")
            sp.dma_start(t[:], nc.t.yT[k * 128:(k + 1) * 128, :])
            yt.append(t)
        load_chunk(0)
        load_chunk(1)
        wv = []
        t = cst.tile([128, C], F16, name="wv0", tag="wv0")
        sp.dma_start(t[:], nc.t.wvT[0:128, :])
        wv.append(t)
        for pc in range(2, NPC):
            load_chunk(pc)
        # Scalar ring (queue idle until the first prologue ACT): wq+ident
        wq = cst.tile([C4, C], F16, name="wq", tag="wq")
        nc.scalar.dma_start(wq[:], nc.t.wq[:])
        ident = cst.tile([128, 128], F16, name="ident", tag="ident")
        nc.scalar.dma_start(ident[:], nc.t.ident[:])
        # GpSimd ring (Pool queue idle until the first store): biases+wv
        bk_sb = cst.tile([C4, 1], F32, name="bk_sb", tag="bk_sb")
        nc.gpsimd.dma_start(bk_sb[:], nc.t.bk[:])
        bqb = cst.tile([C4, K], F16, name="bqb", tag="bqb")
        nc.gpsimd.dma_start(bqb[:], nc.t.bqb[:])
        bvr = cst.tile([1, C], F16, name="bvr", tag="bvr")
        nc.gpsimd.dma_start(bvr[:], nc.t.bv[:])
        for k in range(1, KC):
            t = cst.tile([128, C], F16, name=f"wv{k}", tag=f"wv{k}")
            nc.gpsimd.dma_start(t[:], nc.t.wvT[k * 128:(k + 1) * 128, :])
            wv.append(t)
        s_bc = cst.tile([K, 1], F32, name="s_bc", tag="s_bc")
        nc.gpsimd.dma_start(
            s_bc[:], nc.t.s[:].partition_broadcast(K).squeeze(-1))

        # preload the activation table off the critical path (the first
        # ACT instruction pays a ~1.3us ACT_TABLE_LOAD)
        scr = cst.tile([1, 1], F32, name="scr", tag="scr")
        nc.scalar.activation(out=scr[:], in_=zb[:], func=AF.Exp,
                             bias=zb[:], scale=1.0)

        # ------- PE warmup + fillers: dep-free matmuls on zeros ---------
        # dmy_ps is never read, so these matmuls dispatch the moment the
        # PE is free - they start the clock ramp immediately and later
        # plug pipeline bubbles so the PE never drops out of 2.4 GHz.
        dmy_ps = ps.tile([128, PC], F32, name="dmy_ps", tag="dmy", bufs=1)

        def fill(n):
            for _ in range(n):
                nc.tensor.matmul(dmy_ps[:], dmw[:, 0:128], dmw[:],
                                 start=True, stop=True)

        fill(NWARM)

        # ---------------- prologue: ktb, kq^T, e_b, v16 -----------------
        pro = {}

        def prologue_kq():
            # ktb[o, j] = k[j, o] = sum_c wk[o,c] y[j,c] + bk[o]
            kt_ps = ps.tile([C4, K], F32, name="kt_ps", tag="es", bufs=3)
            for k in range(KC):
                nc.tensor.matmul(kt_ps[:], wkt[k][:], yt[k][:],
                                 start=(k == 0), stop=(k == KC - 1))
            ktb = cst.tile([C4, K], F16, name="ktb", tag="ktb")
            nc.scalar.activation(out=ktb[:], in_=kt_ps[:], func=AF.Identity,
                                 bias=bk_sb[:], scale=1.0)
            fill(3)   # cover the PE's wait for the ktb crossing
            # kqT[c', kc, j] = kq[j, 128*kc+c'] = sum_o wq[o,c] k[j,o]
            kq_ps = ps.tile([128, KC, K], F32, name="kq_ps", tag="es",
                            bufs=3)
            for k in range(KC):
                nc.tensor.matmul(kq_ps[:, k, :],
                                 wq[:, k * 128:(k + 1) * 128], ktb[:],
                                 start=True, stop=True)
            kqT = cst.tile([128, KC, K], F16, name="kqT", tag="kqT")
            nc.scalar.activation(out=kqT[:], in_=kq_ps[:], func=AF.Copy,
                                 scale=1.0)
            # per-key energy bias e_b[j] = sum_o bq[o] k[j,o]
            eb_ps = ps.tile([K, K], F32, name="eb_ps", tag="es", bufs=3)
            nc.tensor.matmul(eb_ps[:], ktb[:], bqb[:], start=True, stop=True)
            e_b = cst.tile([K, 1], F32, name="e_b", tag="e_b")
            nc.scalar.activation(out=e_b[:], in_=eb_ps[:, 0:1], func=AF.Copy,
                                 scale=1.0)
            pro.update(ktb=ktb, kqT=kqT, e_b=e_b)

        def prologue_v():
            # v16[j, c] = s * (sum_cc y[j,cc] wv[c,cc] + bv[c])
            v_ps = ps.tile([K, C], F32, name="v_ps", tag="es", bufs=3)
            for k in range(KC):
                nc.tensor.matmul(v_ps[:], yt[k][:], wv[k][:],
                                 start=(k == 0), stop=False)
            nc.tensor.matmul(v_ps[:], ones1[:], bvr[:], start=False,
                             stop=True)
            v16 = cst.tile([K, C], F16, name="v16", tag="v16")
            nc.scalar.activation(out=v16[:], in_=v_ps[:], func=AF.Copy,
                                 bias=0.0, scale=s_bc[:])
            pro.update(v16=v16)

        # ------------- software-pipelined main loop over column chunks --
        expts = [None] * NPC
        attns = [None] * NPC

        def stage_e(pc):
            e_ps = ps.tile([K, PC], F32, name=f"e_ps{pc}", tag="es", bufs=3)
            for k in range(KC):
                nc.tensor.matmul(e_ps[:], pro['kqT'][:, k, :],
                                 xs[pc][:, k, :],
                                 start=(k == 0), stop=(k == KC - 1))
            ex = wkp.tile([K, PC], BF16, name="ex", tag="ex", bufs=4)
            nc.scalar.activation(out=ex[:], in_=e_ps[:], func=AF.Exp,
                                 bias=pro['e_b'][:], scale=1.0)
            expts[pc] = ex

        def stage_sm(pc):
            s_ps = ps.tile([K, PC], F32, name=f"s_ps{pc}", tag="es", bufs=3)
            nc.tensor.matmul(s_ps[:], ones32[:], expts[pc][:],
                             start=True, stop=True)
            rec = wkp.tile([K, PC], F32, name="rec", tag="rec", bufs=2)
            nc.vector.reciprocal_approx_fast(out=rec[:], in_=s_ps[:])
            at = wkp.tile([K, PC], F16, name="at", tag="at", bufs=3)
            nc.gpsimd.tensor_mul(at[:], expts[pc][:], rec[:])
            attns[pc] = at

        def stage_out(pc):
            # two 2-bank PSUM pair-tiles per chunk sharing a
            # double-buffered tag, so the next chunk's matmuls never wait
            # on this chunk's crossings.
            osb = wkp.tile([128, CT, PC], F16, name="osb", tag="osb", bufs=3)
            # pair A (row-tiles 0,1): the residual x is accumulated on
            # the PE by an identity matmul, so the crossing is a pure
            # Scalar cast
            oa = ps.tile([128, 2, PC], F32, name=f"oa{pc}", tag="o", bufs=2)
            for i in range(2):
                nc.tensor.matmul(oa[:, i, :],
                                 pro['v16'][:, i * 128:(i + 1) * 128],
                                 attns[pc][:], start=True, stop=False)
                nc.tensor.matmul(oa[:, i, :], ident[:], xs[pc][:, i, :],
                                 start=False, stop=True)
            nc.scalar.activation(out=osb[:, 0:2, :], in_=oa[:],
                                 func=AF.Copy, scale=1.0)
            # pair B (row-tiles 2,3): plain matmuls, residual added by
            # the DVE straight from PSUM
            obp = ps.tile([128, 2, PC], F32, name=f"ob{pc}", tag="o",
                          bufs=2)
            for i in range(2):
                ct = 2 + i
                nc.tensor.matmul(obp[:, i, :],
                                 pro['v16'][:, ct * 128:(ct + 1) * 128],
                                 attns[pc][:], start=True, stop=True)
            nc.vector.tensor_add(osb[:, 2:, :], obp[:], xs[pc][:, 2:, :])
            dst = nc.t.out[:, pc * PC:(pc + 1) * PC].rearrange(
                "(k p) f -> p k f", p=128)
            st.dma_start(dst, osb[:])

        prologue_kq()
        prologue_v()
        stage_e(0)
        # skew: exp rides with e; softmax 2 steps behind; out 4 behind.
        # Every cross-engine dependency gets >=1 full step of slack, so
        # the in-order queues never head-block each other.
        for step in range(1, NPC + 4):
            fill(FILL if step < NPC else 2)
            if 0 <= step - 2 < NPC:
                stage_sm(step - 2)
            if 0 <= step - 3 < NPC:
                stage_out(step - 3)
            if step < NPC:
                stage_e(step)


class _T:
    """Attribute access to declared dram params."""
    def __init__(self):
        self.__dict__ = {}


_NC_CACHE = []


def _build():
    if _NC_CACHE:
        return _NC_CACHE[0]
    nc = bacc.Bacc(target_bir_lowering=False)
    nc.t = _T()
    t = nc.t
    t.x = nc.declare_dram_parameter("x", [C, HW], F16, isOutput=False)
    t.yT = nc.declare_dram_parameter("yT", [C, K], F16, isOutput=False)
    t.wq = nc.declare_dram_parameter("wq", [C4, C], F16, isOutput=False)
    t.wkT = nc.declare_dram_parameter("wkT", [C, C4], F16, isOutput=False)
    t.wvT = nc.declare_dram_parameter("wvT", [C, C], F16, isOutput=False)
    t.bqb = nc.declare_dram_parameter("bqb", [C4, K], F16, isOutput=False)
    t.bk = nc.declare_dram_parameter("bk", [C4, 1], F32, isOutput=False)
    t.bv = nc.declare_dram_parameter("bv", [1, C], F16, isOutput=False)
    t.s = nc.declare_dram_parameter("s", [1, 1], F32, isOutput=False)
    t.ident = nc.declare_dram_parameter("ident", [128, 128], F16,
                                        isOutput=False)
    t.out = nc.declare_dram_parameter("out", [C, HW], F16, isOutput=True)
    with tile.TileContext(nc) as tc:
        _emit(nc, tc)
    nc.finalize()
    _NC_CACHE.append(nc)
    return nc


def _in_maps(x, y, wq, bq, wk, bk, wv, bv, scale):
    x16 = np.ascontiguousarray(x, dtype=np.float16).reshape(N, C, HW)
    yT = np.ascontiguousarray(
        np.transpose(y, (0, 2, 1)), dtype=np.float16)
    wq16 = np.ascontiguousarray(wq, dtype=np.float16)
    wkT = np.ascontiguousarray(np.float16(wk).T)
    wvT = np.ascontiguousarray(np.float16(wv).T)
    bqb = np.ascontiguousarray(
        np.broadcast_to(np.float16(bq).reshape(C4, 1), (C4, K)))
    bk32 = np.ascontiguousarray(bk, dtype=np.float32).reshape(C4, 1)
    bv16 = np.ascontiguousarray(bv, dtype=np.float16).reshape(1, C)
    s = np.ascontiguousarray(scale, dtype=np.float32).reshape(1, 1)
    ident = np.eye(128, dtype=np.float16)
    return [
        {
            "x": x16[i], "yT": yT[i], "wq": wq16, "wkT": wkT, "wvT": wvT,
            "bqb": bqb, "bk": bk32, "bv": bv16, "s": s, "ident": ident,
        }
        for i in range(N)
    ]


def _run(inputs, **kwargs):
    nc = _build()
    return run_bass_kernel_spmd(nc, _in_maps(**inputs),
                                core_ids=list(range(N)), **kwargs)


def kernel(**inputs) -> np.ndarray:
    res = _run(inputs)
    out = np.stack([res.results[i]["out"] for i in range(N)])
    return out.reshape(N, C, H, W).astype(np.float32)
